# revision 15
# baseline (speedup 1.0000x reference)
"""Trainium2 Bass kernel for MultiHeadLatentAttention (B=2, T=2048, E=1024, H=16, L=1024).

Math (per reference):
  q = x @ Wq + bq                     -> [B,T,E],  heads on last dim
  k = (x @ Wk + bk) @ Wl + bl        -> [B,T,L]
  v = (x @ Wv + bv) @ Wl + bl        -> [B,T,L]
  attn = softmax(q k^T / sqrt(64))    per (batch, head)
  out = (attn @ v) @ Wo + bo          -> [B,T,E]

Sharding: 16 heads over 8 cores -> 2 heads (128 cols of Wq / Wl / rows of Wo)
per core.  The latent projection is algebraically fused on-device:
Wkl = Wk @ Wl[:, J], so k_head = x @ Wkl + (bk @ Wl[:, J] + bl[J]); same for v.
Each core computes its 2 heads' attention and a partial output (attn_out @
Wo[J, :]); the host sums the 8 partials and adds bo.

On-core layout: everything is computed transposed ("T" = [feature, token])
so the softmax key-dim lands on SBUF partitions and no transposes of the
attention matrix are needed.  Matmuls run as float32r (~fp22 mantissa) at
full PE rate; accumulation is fp32 in PSUM.
"""

import numpy as np

import concourse.bass as bass
import concourse.mybir as mybir
import concourse.tile as tile
from concourse import bacc, bass_utils
from concourse.masks import make_identity

P = 128
E = 1024
NE = E // P          # 8 contraction tiles
B = 2
T = 2048
BT = B * T           # 4096
H_PER_CORE = 2
HD = 64              # head dim
N_CORES = 8
TT = BT // P         # 32 token tiles
F32 = mybir.dt.float32
F32R = mybir.dt.float32r
AF = mybir.ActivationFunctionType
SCALE = 1.0 / 8.0    # 1/sqrt(64)

_CACHED = {}


def _build():
    nc = bacc.Bacc("TRN2", target_bir_lowering=False, debug=False,
                   num_devices=N_CORES)

    xA = nc.dram_tensor("xA", [NE, P, BT], F32R, kind="ExternalInput").ap()
    wq_d = nc.dram_tensor("wq", [NE, P, P], F32R, kind="ExternalInput").ap()
    wl_d = nc.dram_tensor("wl", [NE, P, P], F32R, kind="ExternalInput").ap()
    wkT_d = nc.dram_tensor("wkT", [NE, P, E], F32R, kind="ExternalInput").ap()
    wvT_d = nc.dram_tensor("wvT", [NE, P, E], F32R, kind="ExternalInput").ap()
    wo_d = nc.dram_tensor("wo", [P, E], F32R, kind="ExternalInput").ap()
    bq_d = nc.dram_tensor("bq", [P, 1], F32, kind="ExternalInput").ap()
    bkl_d = nc.dram_tensor("bkl", [P, 1], F32, kind="ExternalInput").ap()
    bvl_d = nc.dram_tensor("bvl", [P, 1], F32, kind="ExternalInput").ap()
    out_d = nc.dram_tensor("out", [BT, E], F32, kind="ExternalOutput").ap()
    out_r = out_d.rearrange("(o p) e -> p o e", p=P)

    with tile.TileContext(nc) as tc:
        _emit(nc, tc, xA, wq_d, wl_d, wkT_d, wvT_d, wo_d,
              bq_d, bkl_d, bvl_d, out_r)

    nc.compile()
    return nc


def _emit(nc, tc, xA, wq_d, wl_d, wkT_d, wvT_d, wo_d,
          bq_d, bkl_d, bvl_d, out_r):
    from contextlib import ExitStack

    with ExitStack() as ctx:
        wpool = ctx.enter_context(tc.tile_pool(name="wpool", bufs=1))
        psum = ctx.enter_context(tc.tile_pool(name="psum", bufs=2, space="PSUM"))

        # persistent SBUF tensors
        wq_sb = wpool.tile([P, NE, P], F32R)
        wkl_sb = wpool.tile([P, NE, P], F32R)
        wvl_sb = wpool.tile([P, NE, P], F32R)
        wo_sb = wpool.tile([P, E], F32R)
        bq_sb = wpool.tile([P, 1], F32)
        bkl_sb = wpool.tile([P, 1], F32)
        bvl_sb = wpool.tile([P, 1], F32)
        id_sb = wpool.tile([P, P], F32)
        qT = wpool.tile([P, BT], F32R)       # [2 heads x 64, tokens]
        kT = wpool.tile([P, BT], F32R)
        # V in [key, feature] layout: [key%128, token_tile, 2x(64 V + 64 ones)]
        # The 64 ones-columns make the PV matmul emit the softmax denominator
        # pre-broadcast into PSUM partitions 64-127 (no partition-broadcast op
        # exists, and matmul N-cycles don't depend on M).
        V_all = wpool.tile([P, TT, 256], F32R)
        aoT = wpool.tile([P, BT], F32R)      # attention output ^T

        nc.sync.dma_start(wq_sb, wq_d.rearrange("o p j -> p o j"))
        nc.sync.dma_start(wo_sb, wo_d)
        nc.sync.dma_start(bq_sb, bq_d)
        nc.sync.dma_start(bkl_sb, bkl_d)
        nc.sync.dma_start(bvl_sb, bvl_d)
        make_identity(nc, id_sb)
        ones_sb = wpool.tile([P, 1], F32)
        nc.any.memset(ones_sb, 1.0)
        nc.vector.tensor_copy(V_all[:, :, 64:128],
                              ones_sb.to_broadcast((P, TT, 64)))
        nc.vector.tensor_copy(V_all[:, :, 192:256],
                              ones_sb.to_broadcast((P, TT, 64)))

        # ---------------- phase 1: fused latent weights --------------------
        # WxlT[j, e] = sum_e2 Wl[e2, j] * Wx[e, e2]  (= (Wx @ Wl[:, J])^T),
        # then PE-transpose into Wxl[e, j] tiles for use as projection lhsT.
        with tc.tile_pool(name="fpool", bufs=2) as fpool:
            wl_sb = fpool.tile([P, NE, P], F32R, tag="wl")
            nc.sync.dma_start(wl_sb, wl_d.rearrange("o p j -> p o j"))
            for (wT_d, wxl_sb) in ((wkT_d, wkl_sb), (wvT_d, wvl_sb)):
                wT_sb = fpool.tile([P, NE, E], F32R, tag="wT")
                nc.sync.dma_start(wT_sb, wT_d.rearrange("o p e -> p o e"))
                wlT_tmp = fpool.tile([P, E], F32, tag="wlT")
                for ch in range(2):
                    ps = psum.tile([P, 512], F32, tag="o")
                    for o2 in range(NE):
                        nc.tensor.matmul(
                            ps,
                            lhsT=wl_sb[:, o2],
                            rhs=wT_sb[:, o2, ch * 512:(ch + 1) * 512],
                            start=(o2 == 0), stop=(o2 == NE - 1))
                    nc.vector.tensor_copy(wlT_tmp[:, ch * 512:(ch + 1) * 512], ps)
                for et in range(NE):
                    pst = psum.tile([P, P], F32, tag="o")
                    nc.tensor.transpose(pst, wlT_tmp[:, et * P:(et + 1) * P], id_sb)
                    nc.vector.tensor_copy(wxl_sb[:, et], pst)

        # ---------------- phase 2: q/k/v projections ------------------------
        # qT/kT/vT[j, t] = sum_e W[e, j] x^T[e, t] + b[j]
        with tc.tile_pool(name="xpool", bufs=3) as xpool, \
             tc.tile_pool(name="vtpool", bufs=1) as vtpool:
            vT = vtpool.tile([P, BT], F32)
            for tcd in range(8):
                ts0 = tcd * 512
                xt = xpool.tile([P, NE, 512], F32R, tag="x")
                nc.sync.dma_start(
                    xt, xA[:, :, ts0:ts0 + 512].rearrange("o p t -> p o t"))
                for (w_sb, dst, b_sb2) in ((wq_sb, qT, bq_sb),
                                           (wkl_sb, kT, bkl_sb),
                                           (wvl_sb, vT, bvl_sb)):
                    ps = psum.tile([P, 512], F32, tag="o")
                    for o in range(NE):
                        nc.tensor.matmul(
                            ps,
                            lhsT=w_sb[:, o],
                            rhs=xt[:, o],
                            start=(o == 0), stop=(o == NE - 1))
                    nc.vector.tensor_scalar_add(dst[:, ts0:ts0 + 512], ps, b_sb2)
                # V tiles ([key, feat]) for this chunk via PE transpose
                for tt in range(4):
                    tg = tcd * 4 + tt
                    pst = psum.tile([P, P], F32, tag="o")
                    nc.tensor.transpose(pst, vT[:, tg * P:(tg + 1) * P], id_sb)
                    nc.vector.tensor_copy(V_all[:, tg, 0:64], pst[:, 0:64])
                    nc.vector.tensor_copy(V_all[:, tg, 128:192], pst[:, 64:128])

        # ---------------- phase 3: attention + output projection ------------
        epool = ctx.enter_context(tc.tile_pool(name="epool", bufs=3))
        spool = ctx.enter_context(tc.tile_pool(name="spool", bufs=2))
        stpool = ctx.enter_context(tc.tile_pool(name="stpool", bufs=2))

        KT = T // P  # 16 key tiles per batch
        GROUP = 3
        for b in range(B):
            for qb in range(4):
                q0 = b * T + qb * 512
                # S^T tiles: [key_tile(128), q(512)]; heads interleaved so the
                # two K=64 matmuls land in distinct PE row-groups (concurrent).
                pairs = [(hl, kt) for kt in range(KT) for hl in (0, 1)]
                psO = {hl: psum.tile([P, 512], F32, tag="o", name=f"psO{hl}")
                       for hl in (0, 1)}
                for g0 in range(0, len(pairs), GROUP):
                    grp = pairs[g0:g0 + GROUP]
                    w = len(grp) * 512
                    psS = psum.tile([P, GROUP * 512], F32, tag="s")
                    for i, (hl, kt) in enumerate(grp):
                        k0 = b * T + kt * P
                        nc.tensor.matmul(
                            psS[:, i * 512:(i + 1) * 512],
                            lhsT=kT[hl * HD:(hl + 1) * HD, k0:k0 + P],
                            rhs=qT[hl * HD:(hl + 1) * HD, q0:q0 + 512],
                            start=True, stop=True)
                    es = epool.tile([P, GROUP * 512], F32R, tag="e")
                    nc.scalar.activation(es[:, :w], psS[:, :w], AF.Exp, scale=SCALE)
                    for i, (hl, kt) in enumerate(grp):
                        nc.tensor.matmul(
                            psO[hl],
                            lhsT=V_all[:, b * KT + kt, hl * P:(hl + 1) * P],
                            rhs=es[:, i * 512:(i + 1) * 512],
                            start=(kt == 0), stop=(kt == KT - 1))
                # normalize: rows 0-63 = unnormalized O^T, rows 64-127 = sum(exp)
                for hl in (0, 1):
                    recb = spool.tile([64, 512], F32, tag="recb")
                    nc.vector.reciprocal(recb, psO[hl][64:128, :])
                    nc.vector.tensor_mul(
                        aoT[hl * HD:(hl + 1) * HD, q0:q0 + 512],
                        psO[hl][0:64, :], recb)
                # partial output projection for these 4 token tiles
                stage = stpool.tile([P, 4, E], F32, tag="st")
                for tt in range(4):
                    t0 = q0 + tt * P
                    for ch in range(2):
                        pso = psum.tile([P, 512], F32, tag="o")
                        nc.tensor.matmul(
                            pso,
                            lhsT=aoT[:, t0:t0 + P],
                            rhs=wo_sb[:, ch * 512:(ch + 1) * 512],
                            start=True, stop=True)
                        nc.vector.tensor_copy(stage[:, tt, ch * 512:(ch + 1) * 512], pso)
                ot = (b * T + qb * 512) // P
                nc.sync.dma_start(out_r[:, ot:ot + 4, :], stage)


def _get_nc():
    if "nc" not in _CACHED:
        _CACHED["nc"] = _build()
    return _CACHED["nc"]


def _in_maps(x, Wq, bq, Wk, bk, Wv, bv, Wl, bl, Wo, bo):
    xf = np.ascontiguousarray(x.reshape(BT, E).T)          # [E, BT]
    xA = xf.reshape(NE, P, BT)
    wkT = np.ascontiguousarray(Wk.T).reshape(NE, P, E)
    wvT = np.ascontiguousarray(Wv.T).reshape(NE, P, E)
    bkl = bk @ Wl + bl                                      # fused latent bias
    bvl = bv @ Wl + bl
    maps = []
    for c in range(N_CORES):
        J = slice(c * P, (c + 1) * P)
        maps.append({
            "xA": xA,
            "wq": np.ascontiguousarray(Wq[:, J]).reshape(NE, P, P),
            "wl": np.ascontiguousarray(Wl[:, J]).reshape(NE, P, P),
            "wkT": wkT,
            "wvT": wvT,
            "wo": np.ascontiguousarray(Wo[J, :]),
            "bq": np.ascontiguousarray(bq[J]).reshape(P, 1),
            "bkl": np.ascontiguousarray(bkl[J]).astype(np.float32).reshape(P, 1),
            "bvl": np.ascontiguousarray(bvl[J]).astype(np.float32).reshape(P, 1),
        })
    return maps


def kernel(x, Wq, bq, Wk, bk, Wv, bv, Wl, bl, Wo, bo, _trace=False, _trace_kwargs=None):
    x, Wq, bq, Wk, bk, Wv, bv, Wl, bl, Wo, bo = [
        np.asarray(a, dtype=np.float32)
        for a in (x, Wq, bq, Wk, bk, Wv, bv, Wl, bl, Wo, bo)]
    nc = _get_nc()
    maps = _in_maps(x, Wq, bq, Wk, bk, Wv, bv, Wl, bl, Wo, bo)
    kwargs = {}
    if _trace:
        kwargs = dict(trace=True, **(_trace_kwargs or {}))
    res = bass_utils.run_bass_kernel_spmd(
        nc, maps, core_ids=list(range(N_CORES)), **kwargs)
    total = np.zeros((BT, E), np.float32)
    for c in range(N_CORES):
        total += res.results[c]["out"]
    total += bo[None, :]
    out = total.reshape(B, T, E)
    _CACHED["last_results"] = res
    return out


# revision 17
# speedup vs baseline: 1.3499x; 1.3499x over previous
"""Trainium2 Bass kernel for MultiHeadLatentAttention (B=2, T=2048, E=1024, H=16, L=1024).

Math (per reference):
  q = x @ Wq + bq                     -> [B,T,E],  heads on last dim
  k = (x @ Wk + bk) @ Wl + bl        -> [B,T,L]
  v = (x @ Wv + bv) @ Wl + bl        -> [B,T,L]
  attn = softmax(q k^T / sqrt(64))    per (batch, head)
  out = (attn @ v) @ Wo + bo          -> [B,T,E]

Sharding: 16 heads over 8 cores -> 2 heads (128 cols of Wq / Wl / rows of Wo)
per core.  The latent projection is algebraically fused on-device:
Wkl = Wk @ Wl[:, J], so k_head = x @ Wkl + (bk @ Wl[:, J] + bl[J]); same for v.
Each core computes its 2 heads' attention and a partial output (attn_out @
Wo[J, :]); the host sums the 8 partials and adds bo.

On-core layout: everything is computed transposed ("T" = [feature, token])
so the softmax key-dim lands on SBUF partitions and no transposes of the
attention matrix are needed.  Matmul operands are bf16
(full PE rate, FWL weight loads); accumulation is fp32 in PSUM.
"""

import numpy as np

import concourse.bass as bass
import concourse.mybir as mybir
import concourse.tile as tile
from concourse import bacc, bass_utils
from concourse.masks import make_identity

P = 128
E = 1024
NE = E // P          # 8 contraction tiles
B = 2
T = 2048
BT = B * T           # 4096
H_PER_CORE = 2
HD = 64              # head dim
N_CORES = 8
TT = BT // P         # 32 token tiles
F32 = mybir.dt.float32
F32R = mybir.dt.float32r
BF16 = mybir.dt.bfloat16
AF = mybir.ActivationFunctionType
SCALE = 1.0 / 8.0    # 1/sqrt(64)

_CACHED = {}


def _build():
    nc = bacc.Bacc("TRN2", target_bir_lowering=False, debug=False,
                   num_devices=N_CORES)

    xA = nc.dram_tensor("xA", [NE, P, BT], BF16, kind="ExternalInput").ap()
    wq_d = nc.dram_tensor("wq", [NE, P, P], BF16, kind="ExternalInput").ap()
    wl_d = nc.dram_tensor("wl", [NE, P, P], BF16, kind="ExternalInput").ap()
    wkT_d = nc.dram_tensor("wkT", [NE, P, E], BF16, kind="ExternalInput").ap()
    wvT_d = nc.dram_tensor("wvT", [NE, P, E], BF16, kind="ExternalInput").ap()
    wo_d = nc.dram_tensor("wo", [P, E], BF16, kind="ExternalInput").ap()
    bq_d = nc.dram_tensor("bq", [P, 1], F32, kind="ExternalInput").ap()
    bkl_d = nc.dram_tensor("bkl", [P, 1], F32, kind="ExternalInput").ap()
    bvl_d = nc.dram_tensor("bvl", [P, 1], F32, kind="ExternalInput").ap()
    out_d = nc.dram_tensor("out", [BT, E], F32, kind="ExternalOutput").ap()
    out_r = out_d.rearrange("(o p) e -> p o e", p=P)

    with tile.TileContext(nc) as tc:
        _emit(nc, tc, xA, wq_d, wl_d, wkT_d, wvT_d, wo_d,
              bq_d, bkl_d, bvl_d, out_r)

    nc.compile()
    return nc


def _emit(nc, tc, xA, wq_d, wl_d, wkT_d, wvT_d, wo_d,
          bq_d, bkl_d, bvl_d, out_r):
    from contextlib import ExitStack

    with ExitStack() as ctx:
        wpool = ctx.enter_context(tc.tile_pool(name="wpool", bufs=1))
        psum = ctx.enter_context(tc.tile_pool(name="psum", bufs=2, space="PSUM"))

        # persistent SBUF tensors
        wq_sb = wpool.tile([P, NE, P], BF16)
        wkl_sb = wpool.tile([P, NE, P], BF16)
        wvl_sb = wpool.tile([P, NE, P], BF16)
        wo_sb = wpool.tile([P, E], BF16)
        bq_sb = wpool.tile([P, 1], F32)
        bkl_sb = wpool.tile([P, 1], F32)
        bvl_sb = wpool.tile([P, 1], F32)
        id_sb = wpool.tile([P, P], BF16)
        qT = wpool.tile([P, BT], BF16)       # [2 heads x 64, tokens]
        kT = wpool.tile([P, BT], BF16)
        # V in [key, feature] layout: [key%128, token_tile, 2x(64 V + 64 ones)]
        # The 64 ones-columns make the PV matmul emit the softmax denominator
        # pre-broadcast into PSUM partitions 64-127 (no partition-broadcast op
        # exists, and matmul N-cycles don't depend on M).
        V_all = wpool.tile([P, TT, 256], BF16)
        aoT = wpool.tile([P, BT], BF16)      # attention output ^T

        nc.sync.dma_start(wq_sb, wq_d.rearrange("o p j -> p o j"))
        nc.sync.dma_start(wo_sb, wo_d)
        nc.sync.dma_start(bq_sb, bq_d)
        nc.sync.dma_start(bkl_sb, bkl_d)
        nc.sync.dma_start(bvl_sb, bvl_d)
        make_identity(nc, id_sb)
        ones_sb = wpool.tile([P, 1], F32)
        nc.any.memset(ones_sb, 1.0)
        nc.vector.tensor_copy(V_all[:, :, 64:128],
                              ones_sb.to_broadcast((P, TT, 64)))
        nc.vector.tensor_copy(V_all[:, :, 192:256],
                              ones_sb.to_broadcast((P, TT, 64)))

        # ---------------- phase 1: fused latent weights --------------------
        # WxlT[j, e] = sum_e2 Wl[e2, j] * Wx[e, e2]  (= (Wx @ Wl[:, J])^T),
        # then PE-transpose into Wxl[e, j] tiles for use as projection lhsT.
        with tc.tile_pool(name="fpool", bufs=2) as fpool:
            wl_sb = fpool.tile([P, NE, P], BF16, tag="wl")
            nc.sync.dma_start(wl_sb, wl_d.rearrange("o p j -> p o j"))
            for (wT_d, wxl_sb) in ((wkT_d, wkl_sb), (wvT_d, wvl_sb)):
                wT_sb = fpool.tile([P, NE, E], BF16, tag="wT")
                nc.sync.dma_start(wT_sb, wT_d.rearrange("o p e -> p o e"))
                wlT_tmp = fpool.tile([P, E], BF16, tag="wlT")
                for ch in range(2):
                    ps = psum.tile([P, 512], F32, tag="o")
                    for o2 in range(NE):
                        nc.tensor.matmul(
                            ps,
                            lhsT=wl_sb[:, o2],
                            rhs=wT_sb[:, o2, ch * 512:(ch + 1) * 512],
                            start=(o2 == 0), stop=(o2 == NE - 1))
                    nc.vector.tensor_copy(wlT_tmp[:, ch * 512:(ch + 1) * 512], ps)
                for et in range(NE):
                    pst = psum.tile([P, P], BF16, tag="o")
                    nc.tensor.transpose(pst, wlT_tmp[:, et * P:(et + 1) * P], id_sb)
                    nc.vector.tensor_copy(wxl_sb[:, et], pst)

        # ---------------- phase 2: q/k/v projections ------------------------
        # qT/kT/vT[j, t] = sum_e W[e, j] x^T[e, t] + b[j]
        with tc.tile_pool(name="xpool", bufs=3) as xpool, \
             tc.tile_pool(name="vtpool", bufs=1) as vtpool:
            vT = vtpool.tile([P, BT], BF16)
            for tcd in range(8):
                ts0 = tcd * 512
                xt = xpool.tile([P, NE, 512], BF16, tag="x")
                nc.sync.dma_start(
                    xt, xA[:, :, ts0:ts0 + 512].rearrange("o p t -> p o t"))
                for (w_sb, dst, b_sb2) in ((wq_sb, qT, bq_sb),
                                           (wkl_sb, kT, bkl_sb),
                                           (wvl_sb, vT, bvl_sb)):
                    ps = psum.tile([P, 512], F32, tag="o")
                    for o in range(NE):
                        nc.tensor.matmul(
                            ps,
                            lhsT=w_sb[:, o],
                            rhs=xt[:, o],
                            start=(o == 0), stop=(o == NE - 1))
                    nc.vector.tensor_scalar_add(dst[:, ts0:ts0 + 512], ps, b_sb2)
                # V tiles ([key, feat]) for this chunk via PE transpose
                for tt in range(4):
                    tg = tcd * 4 + tt
                    pst = psum.tile([P, P], BF16, tag="o")
                    nc.tensor.transpose(pst, vT[:, tg * P:(tg + 1) * P], id_sb)
                    nc.vector.tensor_copy(V_all[:, tg, 0:64], pst[:, 0:64])
                    nc.vector.tensor_copy(V_all[:, tg, 128:192], pst[:, 64:128])

        # ---------------- phase 3: attention + output projection ------------
        epool = ctx.enter_context(tc.tile_pool(name="epool", bufs=3))
        spool = ctx.enter_context(tc.tile_pool(name="spool", bufs=2))
        stpool = ctx.enter_context(tc.tile_pool(name="stpool", bufs=2))

        KT = T // P  # 16 key tiles per batch
        GROUP = 3
        for b in range(B):
            for qb in range(4):
                q0 = b * T + qb * 512
                # S^T tiles: [key_tile(128), q(512)]; heads interleaved so the
                # two K=64 matmuls land in distinct PE row-groups (concurrent).
                pairs = [(hl, kt) for kt in range(KT) for hl in (0, 1)]
                psO = {hl: psum.tile([P, 512], F32, tag="o", name=f"psO{hl}")
                       for hl in (0, 1)}
                for g0 in range(0, len(pairs), GROUP):
                    grp = pairs[g0:g0 + GROUP]
                    w = len(grp) * 512
                    psS = psum.tile([P, GROUP * 512], F32, tag="s")
                    for i, (hl, kt) in enumerate(grp):
                        k0 = b * T + kt * P
                        nc.tensor.matmul(
                            psS[:, i * 512:(i + 1) * 512],
                            lhsT=kT[hl * HD:(hl + 1) * HD, k0:k0 + P],
                            rhs=qT[hl * HD:(hl + 1) * HD, q0:q0 + 512],
                            start=True, stop=True)
                    es = epool.tile([P, GROUP * 512], BF16, tag="e")
                    nc.scalar.activation(es[:, :w], psS[:, :w], AF.Exp, scale=SCALE)
                    for i, (hl, kt) in enumerate(grp):
                        nc.tensor.matmul(
                            psO[hl],
                            lhsT=V_all[:, b * KT + kt, hl * P:(hl + 1) * P],
                            rhs=es[:, i * 512:(i + 1) * 512],
                            start=(kt == 0), stop=(kt == KT - 1))
                # normalize: rows 0-63 = unnormalized O^T, rows 64-127 = sum(exp)
                for hl in (0, 1):
                    recb = spool.tile([64, 512], F32, tag="recb")
                    nc.vector.reciprocal(recb, psO[hl][64:128, :])
                    nc.vector.tensor_mul(
                        aoT[hl * HD:(hl + 1) * HD, q0:q0 + 512],
                        psO[hl][0:64, :], recb)
                # partial output projection for these 4 token tiles
                stage = stpool.tile([P, 4, E], F32, tag="st")
                for tt in range(4):
                    t0 = q0 + tt * P
                    for ch in range(2):
                        pso = psum.tile([P, 512], F32, tag="o")
                        nc.tensor.matmul(
                            pso,
                            lhsT=aoT[:, t0:t0 + P],
                            rhs=wo_sb[:, ch * 512:(ch + 1) * 512],
                            start=True, stop=True)
                        nc.vector.tensor_copy(stage[:, tt, ch * 512:(ch + 1) * 512], pso)
                ot = (b * T + qb * 512) // P
                nc.sync.dma_start(out_r[:, ot:ot + 4, :], stage)


def _get_nc():
    if "nc" not in _CACHED:
        _CACHED["nc"] = _build()
    return _CACHED["nc"]


def _in_maps(x, Wq, bq, Wk, bk, Wv, bv, Wl, bl, Wo, bo):
    import ml_dtypes
    bf16 = ml_dtypes.bfloat16
    xf = np.ascontiguousarray(x.reshape(BT, E).T).astype(bf16)   # [E, BT]
    xA = xf.reshape(NE, P, BT)
    wkT = np.ascontiguousarray(Wk.T).astype(bf16).reshape(NE, P, E)
    wvT = np.ascontiguousarray(Wv.T).astype(bf16).reshape(NE, P, E)
    bkl = bk @ Wl + bl                                      # fused latent bias
    bvl = bv @ Wl + bl
    maps = []
    for c in range(N_CORES):
        J = slice(c * P, (c + 1) * P)
        maps.append({
            "xA": xA,
            "wq": np.ascontiguousarray(Wq[:, J]).astype(bf16).reshape(NE, P, P),
            "wl": np.ascontiguousarray(Wl[:, J]).astype(bf16).reshape(NE, P, P),
            "wkT": wkT,
            "wvT": wvT,
            "wo": np.ascontiguousarray(Wo[J, :]).astype(bf16),
            "bq": np.ascontiguousarray(bq[J]).reshape(P, 1),
            "bkl": np.ascontiguousarray(bkl[J]).astype(np.float32).reshape(P, 1),
            "bvl": np.ascontiguousarray(bvl[J]).astype(np.float32).reshape(P, 1),
        })
    return maps


def kernel(x, Wq, bq, Wk, bk, Wv, bv, Wl, bl, Wo, bo, _trace=False, _trace_kwargs=None):
    x, Wq, bq, Wk, bk, Wv, bv, Wl, bl, Wo, bo = [
        np.asarray(a, dtype=np.float32)
        for a in (x, Wq, bq, Wk, bk, Wv, bv, Wl, bl, Wo, bo)]
    nc = _get_nc()
    maps = _in_maps(x, Wq, bq, Wk, bk, Wv, bv, Wl, bl, Wo, bo)
    kwargs = {}
    if _trace:
        kwargs = dict(trace=True, **(_trace_kwargs or {}))
    res = bass_utils.run_bass_kernel_spmd(
        nc, maps, core_ids=list(range(N_CORES)), **kwargs)
    total = np.zeros((BT, E), np.float32)
    for c in range(N_CORES):
        total += res.results[c]["out"]
    total += bo[None, :]
    out = total.reshape(B, T, E)
    _CACHED["last_results"] = res
    return out


# revision 20
# speedup vs baseline: 1.4040x; 1.0401x over previous
"""Trainium2 Bass kernel for MultiHeadLatentAttention (B=2, T=2048, E=1024, H=16, L=1024).

Math (per reference):
  q = x @ Wq + bq                     -> [B,T,E],  heads on last dim
  k = (x @ Wk + bk) @ Wl + bl        -> [B,T,L]
  v = (x @ Wv + bv) @ Wl + bl        -> [B,T,L]
  attn = softmax(q k^T / sqrt(64))    per (batch, head)
  out = (attn @ v) @ Wo + bo          -> [B,T,E]

Sharding: 16 heads over 8 cores -> 2 heads (128 cols of Wq / Wl / rows of Wo)
per core.  The latent projection is algebraically fused on-device:
Wkl = Wk @ Wl[:, J], so k_head = x @ Wkl + (bk @ Wl[:, J] + bl[J]); same for v.
Each core computes its 2 heads' attention and a partial output (attn_out @
Wo[J, :]); the host sums the 8 partials and adds bo.

On-core layout: everything is computed transposed ("T" = [feature, token])
so the softmax key-dim lands on SBUF partitions and no transposes of the
attention matrix are needed.  Matmul operands are bf16
(full PE rate, FWL weight loads); accumulation is fp32 in PSUM.
"""

import numpy as np

import concourse.bass as bass
import concourse.mybir as mybir
import concourse.tile as tile
from concourse import bacc, bass_utils
from concourse.masks import make_identity

P = 128
E = 1024
NE = E // P          # 8 contraction tiles
B = 2
T = 2048
BT = B * T           # 4096
H_PER_CORE = 2
HD = 64              # head dim
N_CORES = 8
TT = BT // P         # 32 token tiles
F32 = mybir.dt.float32
F32R = mybir.dt.float32r
BF16 = mybir.dt.bfloat16
AF = mybir.ActivationFunctionType
SCALE = 1.0 / 8.0    # 1/sqrt(64)

_CACHED = {}


def _build():
    nc = bacc.Bacc("TRN2", target_bir_lowering=False, debug=False,
                   num_devices=N_CORES)

    xA = nc.dram_tensor("xA", [NE, P, BT], BF16, kind="ExternalInput").ap()
    wq_d = nc.dram_tensor("wq", [NE, P, P], BF16, kind="ExternalInput").ap()
    wl_d = nc.dram_tensor("wl", [NE, P, P], BF16, kind="ExternalInput").ap()
    wkT_d = nc.dram_tensor("wkT", [NE, P, E], BF16, kind="ExternalInput").ap()
    wvT_d = nc.dram_tensor("wvT", [NE, P, E], BF16, kind="ExternalInput").ap()
    wo_d = nc.dram_tensor("wo", [P, E], BF16, kind="ExternalInput").ap()
    bq_d = nc.dram_tensor("bq", [P, 1], F32, kind="ExternalInput").ap()
    bkl_d = nc.dram_tensor("bkl", [P, 1], F32, kind="ExternalInput").ap()
    bvl_d = nc.dram_tensor("bvl", [P, 1], F32, kind="ExternalInput").ap()
    out_d = nc.dram_tensor("out", [BT, E], F32, kind="ExternalOutput").ap()
    out_r = out_d.rearrange("(o p) e -> p o e", p=P)

    with tile.TileContext(nc) as tc:
        _emit(nc, tc, xA, wq_d, wl_d, wkT_d, wvT_d, wo_d,
              bq_d, bkl_d, bvl_d, out_r)

    nc.compile()
    return nc


def _emit(nc, tc, xA, wq_d, wl_d, wkT_d, wvT_d, wo_d,
          bq_d, bkl_d, bvl_d, out_r):
    from contextlib import ExitStack

    with ExitStack() as ctx:
        wpool = ctx.enter_context(tc.tile_pool(name="wpool", bufs=1))
        psum = ctx.enter_context(tc.tile_pool(name="psum", bufs=2, space="PSUM"))
        psumS = ctx.enter_context(tc.tile_pool(name="psumS", bufs=2, space="PSUM"))

        # persistent SBUF tensors
        wq_sb = wpool.tile([P, NE, P], BF16)
        wkl_sb = wpool.tile([P, NE, P], BF16)
        wvl_sb = wpool.tile([P, NE, P], BF16)
        wo_sb = wpool.tile([P, E], BF16)
        bq_sb = wpool.tile([P, 1], F32)
        bkl_sb = wpool.tile([P, 1], F32)
        bvl_sb = wpool.tile([P, 1], F32)
        id_sb = wpool.tile([P, P], BF16)
        qT = wpool.tile([P, BT], BF16)       # [2 heads x 64, tokens]
        kT = wpool.tile([P, BT], BF16)
        # V in [key, feature] layout: [key%128, token_tile, 2x(64 V + 64 ones)]
        # The 64 ones-columns make the PV matmul emit the softmax denominator
        # pre-broadcast into PSUM partitions 64-127 (no partition-broadcast op
        # exists, and matmul N-cycles don't depend on M).
        V_all = wpool.tile([P, TT, 256], BF16)
        aoT = wpool.tile([P, BT], BF16)      # attention output ^T

        nc.sync.dma_start(wq_sb, wq_d.rearrange("o p j -> p o j"))
        nc.sync.dma_start(wo_sb, wo_d)
        nc.sync.dma_start(bq_sb, bq_d)
        nc.sync.dma_start(bkl_sb, bkl_d)
        nc.sync.dma_start(bvl_sb, bvl_d)
        make_identity(nc, id_sb)
        ones_sb = wpool.tile([P, 1], F32)
        nc.any.memset(ones_sb, 1.0)
        nc.vector.tensor_copy(V_all[:, :, 64:128],
                              ones_sb.to_broadcast((P, TT, 64)))
        nc.vector.tensor_copy(V_all[:, :, 192:256],
                              ones_sb.to_broadcast((P, TT, 64)))

        # ---------------- phase 1: fused latent weights --------------------
        # WxlT[j, e] = sum_e2 Wl[e2, j] * Wx[e, e2]  (= (Wx @ Wl[:, J])^T),
        # then PE-transpose into Wxl[e, j] tiles for use as projection lhsT.
        with tc.tile_pool(name="fpool", bufs=2) as fpool:
            wl_sb = fpool.tile([P, NE, P], BF16, tag="wl")
            nc.sync.dma_start(wl_sb, wl_d.rearrange("o p j -> p o j"))
            for (wT_d, wxl_sb) in ((wkT_d, wkl_sb), (wvT_d, wvl_sb)):
                wT_sb = fpool.tile([P, NE, E], BF16, tag="wT")
                nc.sync.dma_start(wT_sb, wT_d.rearrange("o p e -> p o e"))
                wlT_tmp = fpool.tile([P, E], BF16, tag="wlT")
                for ch in range(2):
                    ps = psum.tile([P, 512], F32, tag="o")
                    for o2 in range(NE):
                        nc.tensor.matmul(
                            ps,
                            lhsT=wl_sb[:, o2],
                            rhs=wT_sb[:, o2, ch * 512:(ch + 1) * 512],
                            start=(o2 == 0), stop=(o2 == NE - 1))
                    nc.vector.tensor_copy(wlT_tmp[:, ch * 512:(ch + 1) * 512], ps)
                for et in range(NE):
                    pst = psum.tile([P, P], BF16, tag="o")
                    nc.tensor.transpose(pst, wlT_tmp[:, et * P:(et + 1) * P], id_sb)
                    nc.vector.tensor_copy(wxl_sb[:, et], pst)

        # ---------------- phase 2: q/k/v projections ------------------------
        # qT/kT/vT[j, t] = sum_e W[e, j] x^T[e, t] + b[j]
        with tc.tile_pool(name="xpool", bufs=3) as xpool, \
             tc.tile_pool(name="vtpool", bufs=1) as vtpool:
            vT = vtpool.tile([P, BT], BF16)
            for tcd in range(8):
                ts0 = tcd * 512
                xt = xpool.tile([P, NE, 512], BF16, tag="x")
                nc.scalar.dma_start(
                    xt, xA[:, :, ts0:ts0 + 512].rearrange("o p t -> p o t"))
                for (w_sb, dst, b_sb2) in ((wq_sb, qT, bq_sb),
                                           (wkl_sb, kT, bkl_sb),
                                           (wvl_sb, vT, bvl_sb)):
                    ps = psum.tile([P, 512], F32, tag="o")
                    for o in range(NE):
                        nc.tensor.matmul(
                            ps,
                            lhsT=w_sb[:, o],
                            rhs=xt[:, o],
                            start=(o == 0), stop=(o == NE - 1))
                    nc.vector.tensor_scalar_add(dst[:, ts0:ts0 + 512], ps, b_sb2)
                # V tiles ([key, feat]) for this chunk via PE transpose
                for tt in range(4):
                    tg = tcd * 4 + tt
                    pst = psum.tile([P, P], BF16, tag="o")
                    nc.tensor.transpose(pst, vT[:, tg * P:(tg + 1) * P], id_sb)
                    nc.vector.tensor_copy(V_all[:, tg, 0:64], pst[:, 0:64])
                    nc.vector.tensor_copy(V_all[:, tg, 128:192], pst[:, 64:128])

        # ---------------- phase 3: attention + output projection ------------
        epool = ctx.enter_context(tc.tile_pool(name="epool", bufs=4))
        spool = ctx.enter_context(tc.tile_pool(name="spool", bufs=2))
        stpool = ctx.enter_context(tc.tile_pool(name="stpool", bufs=2))

        KT = T // P  # 16 key tiles per batch
        GROUP = 3
        for b in range(B):
            for qb in range(4):
                q0 = b * T + qb * 512
                # S^T tiles: [key_tile(128), q(512)]; heads interleaved so the
                # two K=64 matmuls land in distinct PE row-groups (concurrent).
                # psS groups 3 pairs (3 PSUM banks, double-buffered) so exp
                # runs 1536 wide; the last group has 2 pairs.
                pairs = [(hl, kt) for kt in range(KT) for hl in (0, 1)]
                psO = {hl: psum.tile([P, 512], F32, tag="o", name=f"psO{hl}")
                       for hl in (0, 1)}
                for g0 in range(0, len(pairs), GROUP):
                    grp = pairs[g0:g0 + GROUP]
                    grp_w = len(grp) * 512
                    psS = psumS.tile([P, GROUP * 512], F32, tag="s")
                    for i, (hl, kt) in enumerate(grp):
                        k0 = b * T + kt * P
                        nc.tensor.matmul(
                            psS[:, i * 512:(i + 1) * 512],
                            lhsT=kT[hl * HD:(hl + 1) * HD, k0:k0 + P],
                            rhs=qT[hl * HD:(hl + 1) * HD, q0:q0 + 512],
                            start=True, stop=True)
                    es = epool.tile([P, GROUP * 512], BF16, tag="e")
                    nc.scalar.activation(es[:, :grp_w], psS[:, :grp_w],
                                         AF.Exp, scale=SCALE)
                    for i, (hl, kt) in enumerate(grp):
                        nc.tensor.matmul(
                            psO[hl],
                            lhsT=V_all[:, b * KT + kt, hl * P:(hl + 1) * P],
                            rhs=es[:, i * 512:(i + 1) * 512],
                            start=(kt == 0), stop=(kt == KT - 1))
                # normalize: rows 0-63 = unnormalized O^T, rows 64-127 =
                # sum(exp).  Copy PSUM->SBUF first so the psO banks free up
                # immediately; the (slow) reciprocal then runs off the PSUM
                # critical path.
                for hl in (0, 1):
                    ub = spool.tile([P, 512], F32, tag="ub")
                    nc.vector.tensor_copy(ub, psO[hl])
                    recb = spool.tile([64, 512], F32, tag="recb")
                    nc.vector.reciprocal(recb, ub[64:128, :])
                    nc.vector.tensor_mul(
                        aoT[hl * HD:(hl + 1) * HD, q0:q0 + 512],
                        ub[0:64, :], recb)
                # partial output projection for these 4 token tiles
                stage = stpool.tile([P, 4, E], F32, tag="st")
                for tt in range(4):
                    t0 = q0 + tt * P
                    for ch in range(2):
                        pso = psum.tile([P, 512], F32, tag="o")
                        nc.tensor.matmul(
                            pso,
                            lhsT=aoT[:, t0:t0 + P],
                            rhs=wo_sb[:, ch * 512:(ch + 1) * 512],
                            start=True, stop=True)
                        nc.vector.tensor_copy(stage[:, tt, ch * 512:(ch + 1) * 512], pso)
                ot = (b * T + qb * 512) // P
                nc.gpsimd.dma_start(out_r[:, ot:ot + 4, :], stage)


def _get_nc():
    if "nc" not in _CACHED:
        _CACHED["nc"] = _build()
    return _CACHED["nc"]


def _in_maps(x, Wq, bq, Wk, bk, Wv, bv, Wl, bl, Wo, bo):
    import ml_dtypes
    bf16 = ml_dtypes.bfloat16
    xf = np.ascontiguousarray(x.reshape(BT, E).T).astype(bf16)   # [E, BT]
    xA = xf.reshape(NE, P, BT)
    wkT = np.ascontiguousarray(Wk.T).astype(bf16).reshape(NE, P, E)
    wvT = np.ascontiguousarray(Wv.T).astype(bf16).reshape(NE, P, E)
    bkl = bk @ Wl + bl                                      # fused latent bias
    bvl = bv @ Wl + bl
    maps = []
    for c in range(N_CORES):
        J = slice(c * P, (c + 1) * P)
        maps.append({
            "xA": xA,
            "wq": np.ascontiguousarray(Wq[:, J]).astype(bf16).reshape(NE, P, P),
            "wl": np.ascontiguousarray(Wl[:, J]).astype(bf16).reshape(NE, P, P),
            "wkT": wkT,
            "wvT": wvT,
            "wo": np.ascontiguousarray(Wo[J, :]).astype(bf16),
            "bq": np.ascontiguousarray(bq[J]).reshape(P, 1),
            "bkl": np.ascontiguousarray(bkl[J]).astype(np.float32).reshape(P, 1),
            "bvl": np.ascontiguousarray(bvl[J]).astype(np.float32).reshape(P, 1),
        })
    return maps


def kernel(x, Wq, bq, Wk, bk, Wv, bv, Wl, bl, Wo, bo, _trace=False, _trace_kwargs=None):
    x, Wq, bq, Wk, bk, Wv, bv, Wl, bl, Wo, bo = [
        np.asarray(a, dtype=np.float32)
        for a in (x, Wq, bq, Wk, bk, Wv, bv, Wl, bl, Wo, bo)]
    nc = _get_nc()
    maps = _in_maps(x, Wq, bq, Wk, bk, Wv, bv, Wl, bl, Wo, bo)
    kwargs = {}
    if _trace:
        kwargs = dict(trace=True, **(_trace_kwargs or {}))
    res = bass_utils.run_bass_kernel_spmd(
        nc, maps, core_ids=list(range(N_CORES)), **kwargs)
    total = np.zeros((BT, E), np.float32)
    for c in range(N_CORES):
        total += res.results[c]["out"]
    total += bo[None, :]
    out = total.reshape(B, T, E)
    _CACHED["last_results"] = res
    return out


# revision 21
# speedup vs baseline: 1.4210x; 1.0121x over previous
"""Trainium2 Bass kernel for MultiHeadLatentAttention (B=2, T=2048, E=1024, H=16, L=1024).

Math (per reference):
  q = x @ Wq + bq                     -> [B,T,E],  heads on last dim
  k = (x @ Wk + bk) @ Wl + bl        -> [B,T,L]
  v = (x @ Wv + bv) @ Wl + bl        -> [B,T,L]
  attn = softmax(q k^T / sqrt(64))    per (batch, head)
  out = (attn @ v) @ Wo + bo          -> [B,T,E]

Sharding: 16 heads over 8 cores -> 2 heads (128 cols of Wq / Wl / rows of Wo)
per core.  The latent projection is algebraically fused on-device:
Wkl = Wk @ Wl[:, J], so k_head = x @ Wkl + (bk @ Wl[:, J] + bl[J]); same for v.
Each core computes its 2 heads' attention and a partial output (attn_out @
Wo[J, :]); the host sums the 8 partials and adds bo.

On-core layout: everything is computed transposed ("T" = [feature, token])
so the softmax key-dim lands on SBUF partitions and no transposes of the
attention matrix are needed.  Matmul operands are bf16
(full PE rate, FWL weight loads); accumulation is fp32 in PSUM.
"""

import numpy as np

import concourse.bass as bass
import concourse.mybir as mybir
import concourse.tile as tile
from concourse import bacc, bass_utils
from concourse.masks import make_identity

P = 128
E = 1024
NE = E // P          # 8 contraction tiles
B = 2
T = 2048
BT = B * T           # 4096
H_PER_CORE = 2
HD = 64              # head dim
N_CORES = 8
TT = BT // P         # 32 token tiles
F32 = mybir.dt.float32
F32R = mybir.dt.float32r
BF16 = mybir.dt.bfloat16
AF = mybir.ActivationFunctionType
SCALE = 1.0 / 8.0    # 1/sqrt(64)

_CACHED = {}


def _build():
    nc = bacc.Bacc("TRN2", target_bir_lowering=False, debug=False,
                   num_devices=N_CORES)

    xA = nc.dram_tensor("xA", [NE, P, BT], BF16, kind="ExternalInput").ap()
    wq_d = nc.dram_tensor("wq", [NE, P, P], BF16, kind="ExternalInput").ap()
    wl_d = nc.dram_tensor("wl", [NE, P, P], BF16, kind="ExternalInput").ap()
    wkT_d = nc.dram_tensor("wkT", [NE, P, E], BF16, kind="ExternalInput").ap()
    wvT_d = nc.dram_tensor("wvT", [NE, P, E], BF16, kind="ExternalInput").ap()
    wo_d = nc.dram_tensor("wo", [P, E], BF16, kind="ExternalInput").ap()
    bq_d = nc.dram_tensor("bq", [P, 1], F32, kind="ExternalInput").ap()
    bkl_d = nc.dram_tensor("bkl", [P, 1], F32, kind="ExternalInput").ap()
    bvl_d = nc.dram_tensor("bvl", [P, 1], F32, kind="ExternalInput").ap()
    out_d = nc.dram_tensor("out", [BT, E], F32, kind="ExternalOutput").ap()
    out_r = out_d.rearrange("(o p) e -> p o e", p=P)

    with tile.TileContext(nc) as tc:
        _emit(nc, tc, xA, wq_d, wl_d, wkT_d, wvT_d, wo_d,
              bq_d, bkl_d, bvl_d, out_r)

    nc.compile()
    return nc


def _emit(nc, tc, xA, wq_d, wl_d, wkT_d, wvT_d, wo_d,
          bq_d, bkl_d, bvl_d, out_r):
    from contextlib import ExitStack

    with ExitStack() as ctx:
        wpool = ctx.enter_context(tc.tile_pool(name="wpool", bufs=1))
        psum = ctx.enter_context(tc.tile_pool(name="psum", bufs=2, space="PSUM"))
        psumS = ctx.enter_context(tc.tile_pool(name="psumS", bufs=3, space="PSUM"))

        # persistent SBUF tensors
        wq_sb = wpool.tile([P, NE, P], BF16)
        wkl_sb = wpool.tile([P, NE, P], BF16)
        wvl_sb = wpool.tile([P, NE, P], BF16)
        wo_sb = wpool.tile([P, E], BF16)
        bq_sb = wpool.tile([P, 1], F32)
        bkl_sb = wpool.tile([P, 1], F32)
        bvl_sb = wpool.tile([P, 1], F32)
        id_sb = wpool.tile([P, P], BF16)
        KT = T // P          # 16 key tiles per batch
        # per-batch tensors so batch-1 projections overlap batch-0 attention
        qT = [wpool.tile([P, T], BF16, name=f"qT{b}") for b in range(B)]
        kT = [wpool.tile([P, T], BF16, name=f"kT{b}") for b in range(B)]
        # V in [key, feature] layout: [key%128, key_tile, 2x(64 V + 64 ones)]
        # The 64 ones-columns make the PV matmul emit the softmax denominator
        # pre-broadcast into PSUM partitions 64-127 (no partition-broadcast op
        # exists, and matmul N-cycles don't depend on M).
        V_all = [wpool.tile([P, KT, 256], BF16, name=f"V{b}") for b in range(B)]
        aoT = [wpool.tile([P, T], BF16, name=f"aoT{b}") for b in range(B)]

        nc.sync.dma_start(wq_sb, wq_d.rearrange("o p j -> p o j"))
        nc.sync.dma_start(wo_sb, wo_d)
        nc.sync.dma_start(bq_sb, bq_d)
        nc.sync.dma_start(bkl_sb, bkl_d)
        nc.sync.dma_start(bvl_sb, bvl_d)
        make_identity(nc, id_sb)
        ones_sb = wpool.tile([P, 1], F32)
        nc.any.memset(ones_sb, 1.0)
        for b in range(B):
            nc.vector.tensor_copy(V_all[b][:, :, 64:128],
                                  ones_sb.to_broadcast((P, KT, 64)))
            nc.vector.tensor_copy(V_all[b][:, :, 192:256],
                                  ones_sb.to_broadcast((P, KT, 64)))

        # ---------------- phase 1: fused latent weights --------------------
        # WxlT[j, e] = sum_e2 Wl[e2, j] * Wx[e, e2]  (= (Wx @ Wl[:, J])^T),
        # then PE-transpose into Wxl[e, j] tiles for use as projection lhsT.
        with tc.tile_pool(name="fpool", bufs=2) as fpool:
            wl_sb = fpool.tile([P, NE, P], BF16, tag="wl")
            nc.sync.dma_start(wl_sb, wl_d.rearrange("o p j -> p o j"))
            for (wT_d, wxl_sb) in ((wkT_d, wkl_sb), (wvT_d, wvl_sb)):
                wT_sb = fpool.tile([P, NE, E], BF16, tag="wT")
                nc.sync.dma_start(wT_sb, wT_d.rearrange("o p e -> p o e"))
                wlT_tmp = fpool.tile([P, E], BF16, tag="wlT")
                for ch in range(2):
                    ps = psum.tile([P, 512], F32, tag="o")
                    for o2 in range(NE):
                        nc.tensor.matmul(
                            ps,
                            lhsT=wl_sb[:, o2],
                            rhs=wT_sb[:, o2, ch * 512:(ch + 1) * 512],
                            start=(o2 == 0), stop=(o2 == NE - 1))
                    nc.vector.tensor_copy(wlT_tmp[:, ch * 512:(ch + 1) * 512], ps)
                for et in range(NE):
                    pst = psumS.tile([P, P], BF16, tag="s")
                    nc.tensor.transpose(pst, wlT_tmp[:, et * P:(et + 1) * P], id_sb)
                    nc.vector.tensor_copy(wxl_sb[:, et], pst)

        # ---------------- phases 2+3 interleaved per batch ------------------
        epool = ctx.enter_context(tc.tile_pool(name="epool", bufs=6))
        spool = ctx.enter_context(tc.tile_pool(name="spool", bufs=2))
        stpool = ctx.enter_context(tc.tile_pool(name="stpool", bufs=2))
        xpool = ctx.enter_context(tc.tile_pool(name="xpool", bufs=3))
        vtpool = ctx.enter_context(tc.tile_pool(name="vtpool", bufs=1))
        GROUP = 2

        for b in range(B):
            # ---- projections for this batch: qT/kT/vT[j, t] = sum_e W x^T + b
            vT = vtpool.tile([P, T], BF16, name=f"vT{b}", tag="vt")
            for tcd in range(4):
                ts0 = tcd * 512
                xt = xpool.tile([P, NE, 512], BF16, tag="x")
                nc.scalar.dma_start(
                    xt, xA[:, :, b * T + ts0:b * T + ts0 + 512]
                    .rearrange("o p t -> p o t"))
                for (w_sb, dst, b_sb2) in ((wq_sb, qT[b], bq_sb),
                                           (wkl_sb, kT[b], bkl_sb),
                                           (wvl_sb, vT, bvl_sb)):
                    ps = psum.tile([P, 512], F32, tag="o")
                    for o in range(NE):
                        nc.tensor.matmul(
                            ps,
                            lhsT=w_sb[:, o],
                            rhs=xt[:, o],
                            start=(o == 0), stop=(o == NE - 1))
                    nc.vector.tensor_scalar_add(dst[:, ts0:ts0 + 512], ps, b_sb2)
                # V tiles ([key, feat]) for this chunk via PE transpose
                for tt in range(4):
                    tg = tcd * 4 + tt
                    pst = psumS.tile([P, P], BF16, tag="s")
                    nc.tensor.transpose(pst, vT[:, tg * P:(tg + 1) * P], id_sb)
                    nc.vector.tensor_copy(V_all[b][:, tg, 0:64], pst[:, 0:64])
                    nc.vector.tensor_copy(V_all[b][:, tg, 128:192], pst[:, 64:128])

            # ---- attention + output projection for this batch
            for qb in range(4):
                q0 = qb * 512
                # S^T tiles: [key_tile(128), q(512)]; heads interleaved so the
                # two K=64 matmuls land in distinct PE row-groups (concurrent).
                # GROUP=2 pairs per PSUM tile (2 banks), triple-buffered, so
                # the exp stream never waits on an S^T group.
                pairs = [(hl, kt) for kt in range(KT) for hl in (0, 1)]
                psO = {hl: psum.tile([P, 512], F32, tag="o", name=f"psO{hl}")
                       for hl in (0, 1)}
                for g0 in range(0, len(pairs), GROUP):
                    grp = pairs[g0:g0 + GROUP]
                    psS = psumS.tile([P, GROUP * 512], F32, tag="s")
                    for i, (hl, kt) in enumerate(grp):
                        nc.tensor.matmul(
                            psS[:, i * 512:(i + 1) * 512],
                            lhsT=kT[b][hl * HD:(hl + 1) * HD, kt * P:(kt + 1) * P],
                            rhs=qT[b][hl * HD:(hl + 1) * HD, q0:q0 + 512],
                            start=True, stop=True)
                    es = epool.tile([P, GROUP * 512], BF16, tag="e")
                    nc.scalar.activation(es, psS, AF.Exp, scale=SCALE)
                    for i, (hl, kt) in enumerate(grp):
                        nc.tensor.matmul(
                            psO[hl],
                            lhsT=V_all[b][:, kt, hl * P:(hl + 1) * P],
                            rhs=es[:, i * 512:(i + 1) * 512],
                            start=(kt == 0), stop=(kt == KT - 1))
                # normalize: rows 0-63 = unnormalized O^T, rows 64-127 =
                # sum(exp).  Copy PSUM->SBUF first so the psO banks free up
                # immediately; the (slow) reciprocal then runs off the PSUM
                # critical path.
                for hl in (0, 1):
                    ub = spool.tile([P, 512], F32, tag="ub")
                    nc.vector.tensor_copy(ub, psO[hl])
                    recb = spool.tile([64, 512], F32, tag="recb")
                    nc.vector.reciprocal(recb, ub[64:128, :])
                    nc.vector.tensor_mul(
                        aoT[b][hl * HD:(hl + 1) * HD, q0:q0 + 512],
                        ub[0:64, :], recb)
                # partial output projection for these 4 token tiles
                stage = stpool.tile([P, 4, E], F32, tag="st")
                for tt in range(4):
                    t0 = q0 + tt * P
                    for ch in range(2):
                        pso = psum.tile([P, 512], F32, tag="o")
                        nc.tensor.matmul(
                            pso,
                            lhsT=aoT[b][:, t0:t0 + P],
                            rhs=wo_sb[:, ch * 512:(ch + 1) * 512],
                            start=True, stop=True)
                        nc.vector.tensor_copy(stage[:, tt, ch * 512:(ch + 1) * 512], pso)
                ot = (b * T + qb * 512) // P
                nc.gpsimd.dma_start(out_r[:, ot:ot + 4, :], stage)


def _get_nc():
    if "nc" not in _CACHED:
        _CACHED["nc"] = _build()
    return _CACHED["nc"]


def _in_maps(x, Wq, bq, Wk, bk, Wv, bv, Wl, bl, Wo, bo):
    import ml_dtypes
    bf16 = ml_dtypes.bfloat16
    xf = np.ascontiguousarray(x.reshape(BT, E).T).astype(bf16)   # [E, BT]
    xA = xf.reshape(NE, P, BT)
    wkT = np.ascontiguousarray(Wk.T).astype(bf16).reshape(NE, P, E)
    wvT = np.ascontiguousarray(Wv.T).astype(bf16).reshape(NE, P, E)
    bkl = bk @ Wl + bl                                      # fused latent bias
    bvl = bv @ Wl + bl
    maps = []
    for c in range(N_CORES):
        J = slice(c * P, (c + 1) * P)
        maps.append({
            "xA": xA,
            "wq": np.ascontiguousarray(Wq[:, J]).astype(bf16).reshape(NE, P, P),
            "wl": np.ascontiguousarray(Wl[:, J]).astype(bf16).reshape(NE, P, P),
            "wkT": wkT,
            "wvT": wvT,
            "wo": np.ascontiguousarray(Wo[J, :]).astype(bf16),
            "bq": np.ascontiguousarray(bq[J]).reshape(P, 1),
            "bkl": np.ascontiguousarray(bkl[J]).astype(np.float32).reshape(P, 1),
            "bvl": np.ascontiguousarray(bvl[J]).astype(np.float32).reshape(P, 1),
        })
    return maps


def kernel(x, Wq, bq, Wk, bk, Wv, bv, Wl, bl, Wo, bo, _trace=False, _trace_kwargs=None):
    x, Wq, bq, Wk, bk, Wv, bv, Wl, bl, Wo, bo = [
        np.asarray(a, dtype=np.float32)
        for a in (x, Wq, bq, Wk, bk, Wv, bv, Wl, bl, Wo, bo)]
    nc = _get_nc()
    maps = _in_maps(x, Wq, bq, Wk, bk, Wv, bv, Wl, bl, Wo, bo)
    kwargs = {}
    if _trace:
        kwargs = dict(trace=True, **(_trace_kwargs or {}))
    res = bass_utils.run_bass_kernel_spmd(
        nc, maps, core_ids=list(range(N_CORES)), **kwargs)
    total = np.zeros((BT, E), np.float32)
    for c in range(N_CORES):
        total += res.results[c]["out"]
    total += bo[None, :]
    out = total.reshape(B, T, E)
    _CACHED["last_results"] = res
    return out


# revision 22
# speedup vs baseline: 1.5693x; 1.1044x over previous
"""Trainium2 Bass kernel for MultiHeadLatentAttention (B=2, T=2048, E=1024, H=16, L=1024).

Math (per reference):
  q = x @ Wq + bq                     -> [B,T,E],  heads on last dim
  k = (x @ Wk + bk) @ Wl + bl        -> [B,T,L]
  v = (x @ Wv + bv) @ Wl + bl        -> [B,T,L]
  attn = softmax(q k^T / sqrt(64))    per (batch, head)
  out = (attn @ v) @ Wo + bo          -> [B,T,E]

Sharding: 16 heads over 8 cores -> 2 heads (128 cols of Wq / Wl / rows of Wo)
per core.  The latent projection is algebraically fused on-device:
Wkl = Wk @ Wl[:, J], so k_head = x @ Wkl + (bk @ Wl[:, J] + bl[J]); same for v.
Each core computes its 2 heads' attention and a partial output (attn_out @
Wo[J, :]); the host sums the 8 partials and adds bo.

On-core layout: everything is computed transposed ("T" = [feature, token])
so the softmax key-dim lands on SBUF partitions and no transposes of the
attention matrix are needed.  Matmul operands are bf16
(full PE rate, FWL weight loads); accumulation is fp32 in PSUM.
"""

import numpy as np

import concourse.bass as bass
import concourse.mybir as mybir
import concourse.tile as tile
from concourse import bacc, bass_utils
from concourse.masks import make_identity

P = 128
E = 1024
NE = E // P          # 8 contraction tiles
B = 2
T = 2048
BT = B * T           # 4096
H_PER_CORE = 2
HD = 64              # head dim
N_CORES = 8
TT = BT // P         # 32 token tiles
F32 = mybir.dt.float32
F32R = mybir.dt.float32r
BF16 = mybir.dt.bfloat16
AF = mybir.ActivationFunctionType
SCALE = 1.0 / 8.0    # 1/sqrt(64)

_CACHED = {}


def _build():
    nc = bacc.Bacc("TRN2", target_bir_lowering=False, debug=False,
                   num_devices=N_CORES)

    xA = nc.dram_tensor("xA", [NE, P, BT], BF16, kind="ExternalInput").ap()
    wq_d = nc.dram_tensor("wq", [NE, P, P], BF16, kind="ExternalInput").ap()
    wl_d = nc.dram_tensor("wl", [NE, P, P], BF16, kind="ExternalInput").ap()
    wkT_d = nc.dram_tensor("wkT", [NE, P, E], BF16, kind="ExternalInput").ap()
    wvT_d = nc.dram_tensor("wvT", [NE, P, E], BF16, kind="ExternalInput").ap()
    wo_d = nc.dram_tensor("wo", [P, E], BF16, kind="ExternalInput").ap()
    bq_d = nc.dram_tensor("bq", [P, 1], F32, kind="ExternalInput").ap()
    bkl_d = nc.dram_tensor("bkl", [P, 1], F32, kind="ExternalInput").ap()
    bvl_d = nc.dram_tensor("bvl", [P, 1], F32, kind="ExternalInput").ap()
    out_d = nc.dram_tensor("out", [BT, E], F32, kind="ExternalOutput").ap()
    out_r = out_d.rearrange("(o p) e -> p o e", p=P)

    with tile.TileContext(nc) as tc:
        _emit(nc, tc, xA, wq_d, wl_d, wkT_d, wvT_d, wo_d,
              bq_d, bkl_d, bvl_d, out_r)

    nc.compile()
    return nc


def _emit(nc, tc, xA, wq_d, wl_d, wkT_d, wvT_d, wo_d,
          bq_d, bkl_d, bvl_d, out_r):
    from contextlib import ExitStack

    with ExitStack() as ctx:
        wpool = ctx.enter_context(tc.tile_pool(name="wpool", bufs=1))
        psum = ctx.enter_context(tc.tile_pool(name="psum", bufs=2, space="PSUM"))
        psumS = ctx.enter_context(tc.tile_pool(name="psumS", bufs=3, space="PSUM"))

        # persistent SBUF tensors
        wq_sb = wpool.tile([P, NE, P], BF16)
        wkl_sb = wpool.tile([P, NE, P], BF16)
        wvl_sb = wpool.tile([P, NE, P], BF16)
        wo_sb = wpool.tile([P, E], BF16)
        bq_sb = wpool.tile([P, 1], F32)
        bkl_sb = wpool.tile([P, 1], F32)
        bvl_sb = wpool.tile([P, 1], F32)
        id_sb = wpool.tile([P, P], BF16)
        KT = T // P          # 16 key tiles per batch
        # per-batch tensors so batch-1 projections overlap batch-0 attention
        qT = [wpool.tile([P, T], BF16, name=f"qT{b}") for b in range(B)]
        kT = [wpool.tile([P, T], BF16, name=f"kT{b}") for b in range(B)]
        # V in [key, feature] layout: [key%128, key_tile, 2x(64 V + 64 ones)]
        # The 64 ones-columns make the PV matmul emit the softmax denominator
        # pre-broadcast into PSUM partitions 64-127 (no partition-broadcast op
        # exists, and matmul N-cycles don't depend on M).
        V_all = [wpool.tile([P, KT, 256], BF16, name=f"V{b}") for b in range(B)]
        aoT = [wpool.tile([P, T], BF16, name=f"aoT{b}") for b in range(B)]

        nc.sync.dma_start(wq_sb, wq_d.rearrange("o p j -> p o j"))
        nc.sync.dma_start(wo_sb, wo_d)
        nc.sync.dma_start(bq_sb, bq_d)
        nc.sync.dma_start(bkl_sb, bkl_d)
        nc.sync.dma_start(bvl_sb, bvl_d)
        make_identity(nc, id_sb)
        for b in range(B):
            nc.vector.memset(V_all[b][:, :, 64:128], 1.0)
            nc.vector.memset(V_all[b][:, :, 192:256], 1.0)

        # ---------------- phase 1: fused latent weights --------------------
        # WxlT[j, e] = sum_e2 Wl[e2, j] * Wx[e, e2]  (= (Wx @ Wl[:, J])^T),
        # then PE-transpose into Wxl[e, j] tiles for use as projection lhsT.
        with tc.tile_pool(name="fpool", bufs=2) as fpool:
            wl_sb = fpool.tile([P, NE, P], BF16, tag="wl")
            nc.sync.dma_start(wl_sb, wl_d.rearrange("o p j -> p o j"))
            for (wT_d, wxl_sb) in ((wkT_d, wkl_sb), (wvT_d, wvl_sb)):
                wT_sb = fpool.tile([P, NE, E], BF16, tag="wT")
                nc.sync.dma_start(wT_sb, wT_d.rearrange("o p e -> p o e"))
                wlT_tmp = fpool.tile([P, E], BF16, tag="wlT")
                for ch in range(2):
                    ps = psum.tile([P, 512], F32, tag="o")
                    for o2 in range(NE):
                        nc.tensor.matmul(
                            ps,
                            lhsT=wl_sb[:, o2],
                            rhs=wT_sb[:, o2, ch * 512:(ch + 1) * 512],
                            start=(o2 == 0), stop=(o2 == NE - 1))
                    nc.vector.tensor_copy(wlT_tmp[:, ch * 512:(ch + 1) * 512], ps)
                for et in range(NE):
                    pst = psumS.tile([P, P], BF16, tag="s")
                    nc.tensor.transpose(pst, wlT_tmp[:, et * P:(et + 1) * P], id_sb)
                    nc.vector.tensor_copy(wxl_sb[:, et], pst)

        # ---------------- phases 2+3 interleaved per batch ------------------
        epool = ctx.enter_context(tc.tile_pool(name="epool", bufs=6))
        spool = ctx.enter_context(tc.tile_pool(name="spool", bufs=2))
        stpool = ctx.enter_context(tc.tile_pool(name="stpool", bufs=2))
        xpool = ctx.enter_context(tc.tile_pool(name="xpool", bufs=3))
        vtpool = ctx.enter_context(tc.tile_pool(name="vtpool", bufs=1))
        GROUP = 2

        for b in range(B):
            # ---- projections for this batch: qT/kT/vT[j, t] = sum_e W x^T + b
            vT = vtpool.tile([P, T], BF16, name=f"vT{b}", tag="vt")
            for tcd in range(4):
                ts0 = tcd * 512
                xt = xpool.tile([P, NE, 512], BF16, tag="x")
                nc.sync.dma_start(
                    xt, xA[:, :, b * T + ts0:b * T + ts0 + 512]
                    .rearrange("o p t -> p o t"))
                for (w_sb, dst, b_sb2) in ((wq_sb, qT[b], bq_sb),
                                           (wkl_sb, kT[b], bkl_sb),
                                           (wvl_sb, vT, bvl_sb)):
                    ps = psum.tile([P, 512], F32, tag="o")
                    for o in range(NE):
                        nc.tensor.matmul(
                            ps,
                            lhsT=w_sb[:, o],
                            rhs=xt[:, o],
                            start=(o == 0), stop=(o == NE - 1))
                    nc.vector.tensor_scalar_add(dst[:, ts0:ts0 + 512], ps, b_sb2)
                # V tiles ([key, feat]) for this chunk via PE transpose
                for tt in range(4):
                    tg = tcd * 4 + tt
                    pst = psumS.tile([P, P], BF16, tag="s")
                    nc.tensor.transpose(pst, vT[:, tg * P:(tg + 1) * P], id_sb)
                    nc.vector.tensor_copy(V_all[b][:, tg, 0:64], pst[:, 0:64])
                    nc.vector.tensor_copy(V_all[b][:, tg, 128:192], pst[:, 64:128])

            # ---- attention + output projection for this batch
            def outproj(qb):
                # partial output projection for qb's 4 token tiles; emitted one
                # iteration late so its PSUM slot allocations sit behind the
                # next iteration's psO accumulators in the FIFO.
                q0 = qb * 512
                stage = stpool.tile([P, 4, E], F32, tag="st", name="stage")
                for tt in range(4):
                    t0 = q0 + tt * P
                    for ch in range(2):
                        pso = psum.tile([P, 512], F32, tag="o", name="pso")
                        nc.tensor.matmul(
                            pso,
                            lhsT=aoT[b][:, t0:t0 + P],
                            rhs=wo_sb[:, ch * 512:(ch + 1) * 512],
                            start=True, stop=True)
                        nc.vector.tensor_copy(stage[:, tt, ch * 512:(ch + 1) * 512], pso)
                ot = (b * T + q0) // P
                nc.gpsimd.dma_start(out_r[:, ot:ot + 4, :], stage)

            prev_qb = None
            for qb in range(4):
                q0 = qb * 512
                # S^T tiles: [key_tile(128), q(512)]; heads interleaved so the
                # two K=64 matmuls land in distinct PE row-groups (concurrent).
                # GROUP=2 pairs per PSUM tile (2 banks), triple-buffered, so
                # the exp stream never waits on an S^T group.
                pairs = [(hl, kt) for kt in range(KT) for hl in (0, 1)]
                psO = {hl: psum.tile([P, 512], F32, tag="o", name=f"psO{hl}")
                       for hl in (0, 1)}
                for g0 in range(0, len(pairs), GROUP):
                    grp = pairs[g0:g0 + GROUP]
                    psS = psumS.tile([P, GROUP * 512], F32, tag="s")
                    for i, (hl, kt) in enumerate(grp):
                        nc.tensor.matmul(
                            psS[:, i * 512:(i + 1) * 512],
                            lhsT=kT[b][hl * HD:(hl + 1) * HD, kt * P:(kt + 1) * P],
                            rhs=qT[b][hl * HD:(hl + 1) * HD, q0:q0 + 512],
                            start=True, stop=True)
                    es = epool.tile([P, GROUP * 512], BF16, tag="e")
                    nc.scalar.activation(es, psS, AF.Exp, scale=SCALE)
                    for i, (hl, kt) in enumerate(grp):
                        nc.tensor.matmul(
                            psO[hl],
                            lhsT=V_all[b][:, kt, hl * P:(hl + 1) * P],
                            rhs=es[:, i * 512:(i + 1) * 512],
                            start=(kt == 0), stop=(kt == KT - 1))
                # normalize: rows 0-63 = unnormalized O^T, rows 64-127 =
                # sum(exp).  Copy PSUM->SBUF first so the psO banks free up
                # immediately; the (slow) reciprocal then runs off the PSUM
                # critical path.
                for hl in (0, 1):
                    ub = spool.tile([P, 512], F32, tag="ub")
                    nc.vector.tensor_copy(ub, psO[hl])
                    recb = spool.tile([64, 512], F32, tag="recb")
                    nc.vector.reciprocal(recb, ub[64:128, :])
                    nc.vector.tensor_mul(
                        aoT[b][hl * HD:(hl + 1) * HD, q0:q0 + 512],
                        ub[0:64, :], recb)
                if prev_qb is not None:
                    outproj(prev_qb)
                prev_qb = qb
            outproj(prev_qb)


def _get_nc():
    if "nc" not in _CACHED:
        _CACHED["nc"] = _build()
    return _CACHED["nc"]


def _in_maps(x, Wq, bq, Wk, bk, Wv, bv, Wl, bl, Wo, bo):
    import ml_dtypes
    bf16 = ml_dtypes.bfloat16
    xf = np.ascontiguousarray(x.reshape(BT, E).T).astype(bf16)   # [E, BT]
    xA = xf.reshape(NE, P, BT)
    wkT = np.ascontiguousarray(Wk.T).astype(bf16).reshape(NE, P, E)
    wvT = np.ascontiguousarray(Wv.T).astype(bf16).reshape(NE, P, E)
    bkl = bk @ Wl + bl                                      # fused latent bias
    bvl = bv @ Wl + bl
    maps = []
    for c in range(N_CORES):
        J = slice(c * P, (c + 1) * P)
        maps.append({
            "xA": xA,
            "wq": np.ascontiguousarray(Wq[:, J]).astype(bf16).reshape(NE, P, P),
            "wl": np.ascontiguousarray(Wl[:, J]).astype(bf16).reshape(NE, P, P),
            "wkT": wkT,
            "wvT": wvT,
            "wo": np.ascontiguousarray(Wo[J, :]).astype(bf16),
            "bq": np.ascontiguousarray(bq[J]).reshape(P, 1),
            "bkl": np.ascontiguousarray(bkl[J]).astype(np.float32).reshape(P, 1),
            "bvl": np.ascontiguousarray(bvl[J]).astype(np.float32).reshape(P, 1),
        })
    return maps


def kernel(x, Wq, bq, Wk, bk, Wv, bv, Wl, bl, Wo, bo, _trace=False, _trace_kwargs=None):
    x, Wq, bq, Wk, bk, Wv, bv, Wl, bl, Wo, bo = [
        np.asarray(a, dtype=np.float32)
        for a in (x, Wq, bq, Wk, bk, Wv, bv, Wl, bl, Wo, bo)]
    nc = _get_nc()
    maps = _in_maps(x, Wq, bq, Wk, bk, Wv, bv, Wl, bl, Wo, bo)
    kwargs = {}
    if _trace:
        kwargs = dict(trace=True, **(_trace_kwargs or {}))
    res = bass_utils.run_bass_kernel_spmd(
        nc, maps, core_ids=list(range(N_CORES)), **kwargs)
    total = np.zeros((BT, E), np.float32)
    for c in range(N_CORES):
        total += res.results[c]["out"]
    total += bo[None, :]
    out = total.reshape(B, T, E)
    _CACHED["last_results"] = res
    return out


# revision 23
# speedup vs baseline: 1.6879x; 1.0756x over previous
"""Trainium2 Bass kernel for MultiHeadLatentAttention (B=2, T=2048, E=1024, H=16, L=1024).

Math (per reference):
  q = x @ Wq + bq                     -> [B,T,E],  heads on last dim
  k = (x @ Wk + bk) @ Wl + bl        -> [B,T,L]
  v = (x @ Wv + bv) @ Wl + bl        -> [B,T,L]
  attn = softmax(q k^T / sqrt(64))    per (batch, head)
  out = (attn @ v) @ Wo + bo          -> [B,T,E]

Sharding: 16 heads over 8 cores -> 2 heads (128 cols of Wq / Wl / rows of Wo)
per core.  The latent projection is algebraically fused on-device:
Wkl = Wk @ Wl[:, J], so k_head = x @ Wkl + (bk @ Wl[:, J] + bl[J]); same for v.
Each core computes its 2 heads' attention and a partial output (attn_out @
Wo[J, :]); the host sums the 8 partials and adds bo.

On-core layout: everything is computed transposed ("T" = [feature, token])
so the softmax key-dim lands on SBUF partitions and no transposes of the
attention matrix are needed.  Matmul operands are bf16
(full PE rate, FWL weight loads); accumulation is fp32 in PSUM.
"""

import numpy as np

import concourse.bass as bass
import concourse.mybir as mybir
import concourse.tile as tile
from concourse import bacc, bass_utils
from concourse.masks import make_identity

P = 128
E = 1024
NE = E // P          # 8 contraction tiles
B = 2
T = 2048
BT = B * T           # 4096
H_PER_CORE = 2
HD = 64              # head dim
N_CORES = 8
TT = BT // P         # 32 token tiles
F32 = mybir.dt.float32
F32R = mybir.dt.float32r
BF16 = mybir.dt.bfloat16
AF = mybir.ActivationFunctionType
SCALE = 1.0 / 8.0    # 1/sqrt(64)

_CACHED = {}


def _build():
    nc = bacc.Bacc("TRN2", target_bir_lowering=False, debug=False,
                   num_devices=N_CORES)

    xA = nc.dram_tensor("xA", [NE, P, BT], BF16, kind="ExternalInput").ap()
    wq_d = nc.dram_tensor("wq", [NE, P, P], BF16, kind="ExternalInput").ap()
    wl_d = nc.dram_tensor("wl", [NE, P, P], BF16, kind="ExternalInput").ap()
    wkT_d = nc.dram_tensor("wkT", [NE, P, E], BF16, kind="ExternalInput").ap()
    wvT_d = nc.dram_tensor("wvT", [NE, P, E], BF16, kind="ExternalInput").ap()
    wo_d = nc.dram_tensor("wo", [P, E], BF16, kind="ExternalInput").ap()
    bq_d = nc.dram_tensor("bq", [P, 1], F32, kind="ExternalInput").ap()
    bkl_d = nc.dram_tensor("bkl", [P, 1], F32, kind="ExternalInput").ap()
    bvl_d = nc.dram_tensor("bvl", [P, 1], F32, kind="ExternalInput").ap()
    out_d = nc.dram_tensor("out", [BT, E], F32, kind="ExternalOutput").ap()
    out_r = out_d.rearrange("(o p) e -> p o e", p=P)

    with tile.TileContext(nc) as tc:
        _emit(nc, tc, xA, wq_d, wl_d, wkT_d, wvT_d, wo_d,
              bq_d, bkl_d, bvl_d, out_r)

    nc.compile()
    return nc


def _emit(nc, tc, xA, wq_d, wl_d, wkT_d, wvT_d, wo_d,
          bq_d, bkl_d, bvl_d, out_r):
    from contextlib import ExitStack

    with ExitStack() as ctx:
        wpool = ctx.enter_context(tc.tile_pool(name="wpool", bufs=1))
        psum = ctx.enter_context(tc.tile_pool(name="psum", bufs=2, space="PSUM"))
        psumS = ctx.enter_context(tc.tile_pool(name="psumS", bufs=3, space="PSUM"))

        # persistent SBUF tensors
        wq_sb = wpool.tile([P, NE, P], BF16)
        wkl_sb = wpool.tile([P, NE, P], BF16)
        wvl_sb = wpool.tile([P, NE, P], BF16)
        wo_sb = wpool.tile([P, E], BF16)
        bq_sb = wpool.tile([P, 1], F32)
        bkl_sb = wpool.tile([P, 1], F32)
        bvl_sb = wpool.tile([P, 1], F32)
        id_sb = wpool.tile([P, P], BF16)
        KT = T // P          # 16 key tiles per batch
        # per-batch tensors so batch-1 projections overlap batch-0 attention
        qT = [wpool.tile([P, T], BF16, name=f"qT{b}") for b in range(B)]
        kT = [wpool.tile([P, T], BF16, name=f"kT{b}") for b in range(B)]
        # V in [key, feature] layout: [key%128, key_tile, 2x(64 V + 64 ones)]
        # The 64 ones-columns make the PV matmul emit the softmax denominator
        # pre-broadcast into PSUM partitions 64-127 (no partition-broadcast op
        # exists, and matmul N-cycles don't depend on M).
        V_all = [wpool.tile([P, KT, 256], BF16, name=f"V{b}") for b in range(B)]
        aoT = [wpool.tile([P, T], BF16, name=f"aoT{b}") for b in range(B)]

        nc.sync.dma_start(wq_sb, wq_d.rearrange("o p j -> p o j"))
        nc.sync.dma_start(wo_sb, wo_d)
        nc.sync.dma_start(bq_sb, bq_d)
        nc.sync.dma_start(bkl_sb, bkl_d)
        nc.sync.dma_start(bvl_sb, bvl_d)
        make_identity(nc, id_sb)
        for b in range(B):
            nc.vector.memset(V_all[b][:, :, 64:128], 1.0)
            nc.vector.memset(V_all[b][:, :, 192:256], 1.0)

        # ---------------- phase 1: fused latent weights --------------------
        # WxlT[j, e] = sum_e2 Wl[e2, j] * Wx[e, e2]  (= (Wx @ Wl[:, J])^T),
        # then PE-transpose into Wxl[e, j] tiles for use as projection lhsT.
        with tc.tile_pool(name="fpool", bufs=2) as fpool:
            wl_sb = fpool.tile([P, NE, P], BF16, tag="wl")
            nc.sync.dma_start(wl_sb, wl_d.rearrange("o p j -> p o j"))
            for (wT_d, wxl_sb) in ((wkT_d, wkl_sb), (wvT_d, wvl_sb)):
                wT_sb = fpool.tile([P, NE, E], BF16, tag="wT")
                nc.sync.dma_start(wT_sb, wT_d.rearrange("o p e -> p o e"))
                wlT_tmp = fpool.tile([P, E], BF16, tag="wlT")
                for ch in range(2):
                    ps = psum.tile([P, 512], F32, tag="o")
                    for o2 in range(NE):
                        nc.tensor.matmul(
                            ps,
                            lhsT=wl_sb[:, o2],
                            rhs=wT_sb[:, o2, ch * 512:(ch + 1) * 512],
                            start=(o2 == 0), stop=(o2 == NE - 1))
                    nc.vector.tensor_copy(wlT_tmp[:, ch * 512:(ch + 1) * 512], ps)
                for et in range(NE):
                    pst = psumS.tile([P, P], BF16, tag="s")
                    nc.tensor.transpose(pst, wlT_tmp[:, et * P:(et + 1) * P], id_sb)
                    nc.vector.tensor_copy(wxl_sb[:, et], pst)

        # ---------------- phases 2+3 interleaved per batch ------------------
        epool = ctx.enter_context(tc.tile_pool(name="epool", bufs=6))
        spool = ctx.enter_context(tc.tile_pool(name="spool", bufs=2))
        stpool = ctx.enter_context(tc.tile_pool(name="stpool", bufs=2))
        xpool = ctx.enter_context(tc.tile_pool(name="xpool", bufs=3))
        vtpool = ctx.enter_context(tc.tile_pool(name="vtpool", bufs=1))
        GROUP = 2

        def proj_chunk(b, vT, tcd):
            # qT/kT/vT[j, t] = sum_e W[e, j] x^T[e, t] + bias[j]
            ts0 = tcd * 512
            xt = xpool.tile([P, NE, 512], BF16, tag="x", name="xt")
            nc.sync.dma_start(
                xt, xA[:, :, b * T + ts0:b * T + ts0 + 512]
                .rearrange("o p t -> p o t"))
            for (w_sb, dst, b_sb2) in ((wq_sb, qT[b], bq_sb),
                                       (wkl_sb, kT[b], bkl_sb),
                                       (wvl_sb, vT, bvl_sb)):
                ps = psum.tile([P, 512], F32, tag="o", name="ps")
                for o in range(NE):
                    nc.tensor.matmul(
                        ps,
                        lhsT=w_sb[:, o],
                        rhs=xt[:, o],
                        start=(o == 0), stop=(o == NE - 1))
                nc.vector.tensor_scalar_add(dst[:, ts0:ts0 + 512], ps, b_sb2)
            # V tiles ([key, feat]) for this chunk via PE transpose
            for tt in range(4):
                tg = tcd * 4 + tt
                pst = psumS.tile([P, P], BF16, tag="s", name="pst")
                nc.tensor.transpose(pst, vT[:, tg * P:(tg + 1) * P], id_sb)
                nc.vector.tensor_copy(V_all[b][:, tg, 0:64], pst[:, 0:64])
                nc.vector.tensor_copy(V_all[b][:, tg, 128:192], pst[:, 64:128])

        vT_t = [vtpool.tile([P, T], BF16, name=f"vT{b}", tag=f"vt{b}")
                for b in range(B)]
        for tcd in range(4):
            proj_chunk(0, vT_t[0], tcd)

        for b in range(B):
            # ---- attention + output projection for this batch; batch-1's
            # projection chunks are emitted between batch-0's attention
            # iterations so the in-order engines fill wait-bubbles with them.
            def outproj(qb):
                # partial output projection for qb's 4 token tiles; emitted one
                # iteration late so its PSUM slot allocations sit behind the
                # next iteration's psO accumulators in the FIFO.
                q0 = qb * 512
                stage = stpool.tile([P, 4, E], F32, tag="st", name="stage")
                for tt in range(4):
                    t0 = q0 + tt * P
                    for ch in range(2):
                        pso = psum.tile([P, 512], F32, tag="o", name="pso")
                        nc.tensor.matmul(
                            pso,
                            lhsT=aoT[b][:, t0:t0 + P],
                            rhs=wo_sb[:, ch * 512:(ch + 1) * 512],
                            start=True, stop=True)
                        nc.vector.tensor_copy(stage[:, tt, ch * 512:(ch + 1) * 512], pso)
                ot = (b * T + q0) // P
                nc.gpsimd.dma_start(out_r[:, ot:ot + 4, :], stage)

            prev_qb = None
            for qb in range(4):
                if b == 0:
                    proj_chunk(1, vT_t[1], qb)
                q0 = qb * 512
                # S^T tiles: [key_tile(128), q(512)]; heads interleaved so the
                # two K=64 matmuls land in distinct PE row-groups (concurrent).
                # GROUP=2 pairs per PSUM tile (2 banks), triple-buffered, so
                # the exp stream never waits on an S^T group.
                pairs = [(hl, kt) for kt in range(KT) for hl in (0, 1)]
                psO = {hl: psum.tile([P, 512], F32, tag="o", name=f"psO{hl}")
                       for hl in (0, 1)}
                for g0 in range(0, len(pairs), GROUP):
                    grp = pairs[g0:g0 + GROUP]
                    psS = psumS.tile([P, GROUP * 512], F32, tag="s")
                    for i, (hl, kt) in enumerate(grp):
                        nc.tensor.matmul(
                            psS[:, i * 512:(i + 1) * 512],
                            lhsT=kT[b][hl * HD:(hl + 1) * HD, kt * P:(kt + 1) * P],
                            rhs=qT[b][hl * HD:(hl + 1) * HD, q0:q0 + 512],
                            start=True, stop=True)
                    es = epool.tile([P, GROUP * 512], BF16, tag="e")
                    nc.scalar.activation(es, psS, AF.Exp, scale=SCALE)
                    for i, (hl, kt) in enumerate(grp):
                        nc.tensor.matmul(
                            psO[hl],
                            lhsT=V_all[b][:, kt, hl * P:(hl + 1) * P],
                            rhs=es[:, i * 512:(i + 1) * 512],
                            start=(kt == 0), stop=(kt == KT - 1))
                # normalize: rows 0-63 = unnormalized O^T, rows 64-127 =
                # sum(exp).  Copy PSUM->SBUF first so the psO banks free up
                # immediately; the (slow) reciprocal then runs off the PSUM
                # critical path.
                for hl in (0, 1):
                    ub = spool.tile([P, 512], F32, tag="ub")
                    nc.vector.tensor_copy(ub, psO[hl])
                    recb = spool.tile([64, 512], F32, tag="recb")
                    nc.vector.reciprocal_approx_fast(recb, ub[64:128, :])
                    nc.vector.tensor_mul(
                        aoT[b][hl * HD:(hl + 1) * HD, q0:q0 + 512],
                        ub[0:64, :], recb)
                if prev_qb is not None:
                    outproj(prev_qb)
                prev_qb = qb
            outproj(prev_qb)


def _get_nc():
    if "nc" not in _CACHED:
        _CACHED["nc"] = _build()
    return _CACHED["nc"]


def _in_maps(x, Wq, bq, Wk, bk, Wv, bv, Wl, bl, Wo, bo):
    import ml_dtypes
    bf16 = ml_dtypes.bfloat16
    xf = np.ascontiguousarray(x.reshape(BT, E).T).astype(bf16)   # [E, BT]
    xA = xf.reshape(NE, P, BT)
    wkT = np.ascontiguousarray(Wk.T).astype(bf16).reshape(NE, P, E)
    wvT = np.ascontiguousarray(Wv.T).astype(bf16).reshape(NE, P, E)
    bkl = bk @ Wl + bl                                      # fused latent bias
    bvl = bv @ Wl + bl
    maps = []
    for c in range(N_CORES):
        J = slice(c * P, (c + 1) * P)
        maps.append({
            "xA": xA,
            "wq": np.ascontiguousarray(Wq[:, J]).astype(bf16).reshape(NE, P, P),
            "wl": np.ascontiguousarray(Wl[:, J]).astype(bf16).reshape(NE, P, P),
            "wkT": wkT,
            "wvT": wvT,
            "wo": np.ascontiguousarray(Wo[J, :]).astype(bf16),
            "bq": np.ascontiguousarray(bq[J]).reshape(P, 1),
            "bkl": np.ascontiguousarray(bkl[J]).astype(np.float32).reshape(P, 1),
            "bvl": np.ascontiguousarray(bvl[J]).astype(np.float32).reshape(P, 1),
        })
    return maps


def kernel(x, Wq, bq, Wk, bk, Wv, bv, Wl, bl, Wo, bo, _trace=False, _trace_kwargs=None):
    x, Wq, bq, Wk, bk, Wv, bv, Wl, bl, Wo, bo = [
        np.asarray(a, dtype=np.float32)
        for a in (x, Wq, bq, Wk, bk, Wv, bv, Wl, bl, Wo, bo)]
    nc = _get_nc()
    maps = _in_maps(x, Wq, bq, Wk, bk, Wv, bv, Wl, bl, Wo, bo)
    kwargs = {}
    if _trace:
        kwargs = dict(trace=True, **(_trace_kwargs or {}))
    res = bass_utils.run_bass_kernel_spmd(
        nc, maps, core_ids=list(range(N_CORES)), **kwargs)
    total = np.zeros((BT, E), np.float32)
    for c in range(N_CORES):
        total += res.results[c]["out"]
    total += bo[None, :]
    out = total.reshape(B, T, E)
    _CACHED["last_results"] = res
    return out
